# revision 1
# baseline (speedup 1.0000x reference)
"""DUPLEX GAT on trn2 — kernel builder + host glue.

Design:
  - Nodes permuted into NW windows of 128 (degree-balanced), padded to N_pad.
  - Per core c the node tables are ROTATED so that core-local dst windows are
    rows [0, wpc*128) of its private g/er tables -> one SPMD program, all
    per-core variation lives in input data.
  - Phase A (per conv, both stacks): g = x @ (W @ T_bd) node-major table in
    DRAM (T_bd = per-head [attn_l | orth-complement] basis so el rides in g
    cols h*32), er table for own windows.
  - Phase B: per dst-window, per src-half group (int16 gather limit):
    dma_gather g rows by src; S / ST one-hot via iota compares; er broadcast
    to edges via ST matmul; p = exp(leakyrelu(el+er)); c = p*w;
    Z *= c (per head); scatter: num'T += Z_t.T @ S_t, den += p_t.T @ S_t in
    PSUM; epilogue: num'/den, un-transform by T_bd^-1 (+ mean over heads for
    layer 1), bias (+ReLU layer 0).
  - Softmax max-subtraction dropped (exact cancellation; logits are O(3)).
"""
import sys
sys.path.insert(0, '/opt/trn_rl_repo')
from dataclasses import dataclass

import numpy as np

import concourse.bass as bass
import concourse.bacc as bacc
import concourse.tile as tile
from concourse import mybir, library_config

F32 = mybir.dt.float32
I16 = mybir.dt.int16
P = 128
NEG = 0.2


def _patch_drain_split():
    """The installed walrus rejects >1 sem wait on the kernel-tail Drain;
    split the waits across a chain of drains."""
    import bass_rust
    from concourse.tile import ScopedClock

    def patched(self, tick_clock, wait_clock):
        nc = self.nc
        drain_inst = nc.sync.drain()
        wait_clock.add_sem_waits(
            drain_inst.ins, ScopedClock({None: tick_clock.global_clock}))
        si = drain_inst.ins.sync_info
        waits = list(si.on_wait) if si is not None else []
        if len(waits) > 1:
            si.on_wait = waits[:1]
            for i in range(1, len(waits)):
                d2 = nc.sync.drain()
                d2.ins.sync_info = bass_rust.SyncInfo(
                    on_wait=waits[i : i + 1], on_update=[])
        nc.all_engine_barrier()
        popped = nc._tile_sem_poison_stack.pop()
        assert popped is self._sem_poison
        nc.clear_and_free_semaphores(list(self.sems.allocated().values()))
        nc.all_engine_barrier()

    tile.TileContext._drain_and_barrier = patched


_patch_drain_split()


def _patch_loud_ncc():
    """Surface compile-hook exceptions (swallowed by the PJRT plugin)."""
    import traceback
    from concourse import bass2jax
    if getattr(bass2jax, "_loud_ncc", False):
        return
    bass2jax._loud_ncc = True
    orig = bass2jax.neuronx_cc_hook

    def logged(*a, **k):
        try:
            return orig(*a, **k)
        except BaseException:
            with open("/tmp/ncc_hook_err.log", "a") as f:
                f.write(traceback.format_exc() + "\n")
            raise

    bass2jax.neuronx_cc_hook = logged


_patch_loud_ncc()


@dataclass
class Cfg:
    n_nodes: int = 50000
    n_edges: int = 800000
    n_cores: int = 8
    wpc: int = 49           # windows per core
    split: int = 32768      # int16 gather split (rows per table half A)
    heads: int = 4
    fdim: int = 32
    in_dim: int = 128
    gbatch: int = 0         # tiles per gather call (0 = whole group)
    bf16: bool = False      # bf16 scatter matmuls (S/Z/p), er path stays fp32

    @property
    def nw(self):
        return self.n_cores * self.wpc

    @property
    def n_pad(self):
        return self.nw * P


# ----------------------------------------------------------------- host prep

def balance_windows(deg, cfg):
    """Assign nodes (incl pad) to windows, balancing total degree via LPT."""
    import heapq
    n_pad, nw = cfg.n_pad, cfg.nw
    degp = np.zeros(n_pad, np.int64)
    degp[: len(deg)] = deg
    order = np.argsort(-degp, kind="stable")
    heap = [(0, w, 0) for w in range(nw)]  # (load, window, count)
    heapq.heapify(heap)
    win_of = np.empty(n_pad, np.int32)
    slot_of = np.empty(n_pad, np.int32)
    pending = []  # windows that reached 128
    for node in order:
        load, w, cnt = heapq.heappop(heap)
        win_of[node] = w
        slot_of[node] = cnt
        cnt += 1
        if cnt < P:
            heapq.heappush(heap, (load + degp[node], w, cnt))
    new_id = win_of.astype(np.int64) * P + slot_of
    return new_id  # (n_pad,) position of each (padded) node id


def wrap_idx(idx):
    """(n,) -> (128, n//16) int16 wrapped layout for dma_gather."""
    n = len(idx)
    blk = np.asarray(idx, np.int16).reshape(n // 16, 16).T
    return np.tile(blk, (8, 1))


def prep_graph(src, dst, cfg):
    """Build per-core gather/scatter tables. Returns dict."""
    c = cfg
    deg = np.bincount(dst, minlength=c.n_nodes)
    new_id = balance_windows(deg, c)              # old(+pad) -> new position
    node_at = np.full(c.n_pad, -1, np.int64)      # new position -> old id
    node_at[new_id] = np.arange(c.n_pad)

    src_n = new_id[src]
    dst_n = new_id[dst]
    order = np.argsort(dst_n, kind="stable")
    src_s, dst_s = src_n[order], dst_n[order]
    win_s = dst_s // P
    # edge ranges per window
    bounds = np.searchsorted(win_s, np.arange(c.nw + 1))

    # per-core rotated row of a (new-space) node position, per core
    # rot_row(core, pos) = ((pos//P - core*wpc) % nw)*P + pos%P
    def rot_rows(core, pos):
        return ((pos // P - core * c.wpc) % c.nw) * P + pos % P

    # first pass: per-window group sizes in each core's rotation
    # group A iff rot_row < split. rotation differs per core *only* via the
    # window part of src; sizes therefore differ per core.
    TA = TB = 0
    grp_masks = {}
    for core in range(c.n_cores):
        for wl in range(c.wpc):
            g = core * c.wpc + wl
            lo, hi = bounds[g], bounds[g + 1]
            rr = rot_rows(core, src_s[lo:hi])
            mA = rr < c.split
            grp_masks[(core, wl)] = (lo, hi, rr, mA)
            nA = int(mA.sum())
            nB = int((hi - lo) - nA)
            TA = max(TA, -(-nA // P))
            TB = max(TB, -(-nB // P))
    TA = max(TA, 1)
    TB = max(TB, 1)
    T = TA + TB

    nco, wpc = c.n_cores, c.wpc
    idxA = np.zeros((nco, wpc, P, TA * 8), np.int16)
    idxB = np.zeros((nco, wpc, P, TB * 8), np.int16)
    dstmb = np.full((nco, wpc, P, T), -1000.0, np.float32)
    dstrow = np.full((nco, wpc, 1, T * P), -1000.0, np.float32)
    epos = np.full((nco, wpc, T * P), -1, np.int64)  # orig edge slot (sorted order)

    for core in range(nco):
        for wl in range(wpc):
            lo, hi, rr, mA = grp_masks[(core, wl)]
            iA = np.where(mA)[0]
            iB = np.where(~mA)[0]
            la = np.zeros(TA * P, np.int64)   # gather rows group A (pad->0)
            lb = np.zeros(TB * P, np.int64)
            la[: len(iA)] = rr[iA]
            lb[: len(iB)] = rr[iB] - c.split
            idxA[core, wl] = wrap_idx(la)
            idxB[core, wl] = wrap_idx(lb)
            dmb = np.full(T * P, -1000.0, np.float32)
            dmb[: len(iA)] = (dst_s[lo:hi][iA] % P).astype(np.float32)
            dmb[TA * P : TA * P + len(iB)] = (dst_s[lo:hi][iB] % P).astype(np.float32)
            dstmb[core, wl] = dmb.reshape(T, P).T
            dstrow[core, wl, 0] = dmb
            ep = np.full(T * P, -1, np.int64)
            ep[: len(iA)] = lo + iA
            ep[TA * P : TA * P + len(iB)] = lo + iB
            epos[core, wl] = ep

    return dict(
        new_id=new_id, node_at=node_at, order=order, TA=TA, TB=TB,
        idxA=idxA, idxB=idxB, dstmb=dstmb, dstrow=dstrow, epos=epos,
    )


def edge_w_tables(g, w_edge, cfg):
    """Per-core per-window padded edge-weight tables (nc, wpc, P, T)."""
    ws = np.asarray(w_edge)[g["order"]]
    ep = g["epos"]
    out = np.where(ep >= 0, ws[np.clip(ep, 0, None)], 0.0).astype(np.float32)
    n, w, TP = out.shape
    T = TP // P
    return out.reshape(n, w, T, P).transpose(0, 1, 3, 2).copy()


def prep_conv(W, al, ar, b, mean_heads, cfg):
    """Derived weights for one conv. Returns dict of f32 arrays."""
    H, F = cfg.heads, cfg.fdim
    W = np.asarray(W, np.float64)
    al = np.asarray(al, np.float64)
    ar = np.asarray(ar, np.float64)
    b = np.asarray(b, np.float64)
    T_bd = np.zeros((H * F, H * F))
    for h in range(H):
        a = al[h]
        M = np.concatenate([a[:, None], np.eye(F)[:, : F - 1]], 1)
        Q, _ = np.linalg.qr(M)
        blk = np.concatenate([a[:, None], Q[:, 1:]], 1)
        T_bd[h * F : (h + 1) * F, h * F : (h + 1) * F] = blk
    Tinv = np.linalg.inv(T_bd)
    R_bd = np.zeros((H * F, H))
    for h in range(H):
        R_bd[h * F : (h + 1) * F, h] = ar[h]
    out = dict(
        Wg=np.concatenate([W @ T_bd, W @ R_bd], 1).astype(np.float32),
    )
    if mean_heads:
        Mm = np.zeros((H * F, F))
        for h in range(H):
            Mm[h * F : (h + 1) * F] = np.eye(F) / H
        out["Tinv"] = (Tinv @ Mm).astype(np.float32)          # (128, 32)
        out["bcol"] = (b.reshape(H, F).mean(0))[:, None].astype(np.float32)
    else:
        out["Tinv"] = Tinv.astype(np.float32)                  # (128, 128)
        out["bcol"] = b[:, None].astype(np.float32)            # (128, 1)
    return out


def consts_np(cfg):
    H, F = cfg.heads, cfg.fdim
    iotab = np.tile(np.arange(P, dtype=np.float32)[None, :], (P, 1))
    iotac = np.arange(P, dtype=np.float32)[:, None]
    ones1 = np.ones((1, P), np.float32)
    H2 = np.zeros((2, 2 * H, H * F), np.float32)  # [stack, 8, 128]
    for s in range(2):
        for h in range(H):
            H2[s, s * H + h, h * F : (h + 1) * F] = 1.0
    return dict(iotab=iotab, iotac=iotac, ones1=ones1, H2am=H2[0], H2ph=H2[1])


def rotate_tiles(x_tiled, core, cfg):
    """x_tiled (nw,128,128) -> rotated copy for `core`."""
    rot = np.roll(np.arange(cfg.nw), -core * cfg.wpc)
    return np.ascontiguousarray(x_tiled[rot])


def to_xT_tiled(x, g, cfg):
    """x (n_nodes, D) -> permuted transposed tiles (nw, D, 128) f32."""
    n_pad = cfg.n_pad
    D = x.shape[1]
    xp = np.zeros((n_pad, D), np.float32)
    real = g["node_at"] >= 0
    idx = g["node_at"][real]
    keep = idx < cfg.n_nodes
    xp[np.where(real)[0][keep]] = np.asarray(x, np.float32)[idx[keep]]
    return np.ascontiguousarray(xp.reshape(cfg.nw, P, D).transpose(0, 2, 1))


# ------------------------------------------------------------ layer program

def build_layer(cfg, TA, TB, last, debug_level=3):
    """One Bass program: both stacks of one GAT layer. Returns (nc, io)."""
    c = cfg
    T = TA + TB
    HF = c.heads * c.fdim            # 128
    OC = c.fdim if last else HF      # output channels per node
    nc = bacc.Bacc("TRN2", target_bir_lowering=False, debug=False)

    inp = {}

    def dram_in(name, shape, dt=F32):
        inp[name] = nc.dram_tensor(name, list(shape), dt, kind="ExternalInput")
        return inp[name]

    xT = {s: dram_in(f"xT_{s}", (c.nw, c.in_dim, P)) for s in ("am", "ph")}
    Wg = {s: dram_in(f"Wg_{s}", (c.in_dim, HF + c.heads)) for s in ("am", "ph")}
    Tinv = {s: dram_in(f"Tinv_{s}", (HF, OC)) for s in ("am", "ph")}
    bcol = {s: dram_in(f"bcol_{s}", (OC, 1)) for s in ("am", "ph")}
    H2 = {s: dram_in(f"H2_{s}", (2 * c.heads, HF)) for s in ("am", "ph")}
    iotab_d = dram_in("iotab", (P, P))
    iotac_d = dram_in("iotac", (P, 1))
    ones_d = dram_in("ones1", (1, P))
    idxA_d = dram_in("idxA", (c.wpc, P, TA * 8), I16)
    idxB_d = dram_in("idxB", (c.wpc, P, TB * 8), I16)
    dstmb_d = dram_in("dstmb", (c.wpc, P, T))
    dstrow_d = dram_in("dstrow", (c.wpc, 1, T * P))
    wtab = {s: dram_in(f"wtab_{s}", (c.wpc, P, T)) for s in ("am", "ph")}

    out_t = {
        s: nc.dram_tensor(f"out_{s}", [c.wpc, OC, P], F32, kind="ExternalOutput")
        for s in ("am", "ph")
    }

    stacks = ("am", "ph")

    with tile.TileContext(nc) as tc:
        with (
            tc.tile_pool(name="dram", bufs=1, space="DRAM") as dpool,
            tc.tile_pool(name="const", bufs=1) as cpool,
        ):
            gtab = {s: dpool.tile([c.n_pad, HF], F32, name=f"gtab{s}", tag=f"gtab{s}") for s in stacks}
            ertab = dpool.tile([c.wpc * P, 2 * c.heads], F32, name="ertab")

            ct = {}
            for nm, hd, sh in [
                ("iotab", iotab_d, (P, P)), ("iotac", iotac_d, (P, 1)),
                ("ones", ones_d, (1, P)),
            ]:
                ct[nm] = cpool.tile(list(sh), F32, name=f"ct_{nm}")
                nc.sync.dma_start(ct[nm][:], hd[:])
            for s in stacks:
                for nm, hd in [("Wg", Wg[s]), ("Tinv", Tinv[s]),
                               ("bcol", bcol[s]), ("H2", H2[s])]:
                    t = cpool.tile(list(hd.shape), F32, name=f"ct_{nm}_{s}")
                    nc.sync.dma_start(t[:], hd[:])
                    ct[(nm, s)] = t

            # ---------------- phase A: g / er tables ----------------
            CH = next(k for k in (7, 4, 2, 1) if c.wpc % k == 0 and c.nw % k == 0)
            with (
                tc.tile_pool(name="pa_x", bufs=3) as pax,
                tc.tile_pool(name="pa_g", bufs=3) as pag,
                tc.tile_pool(name="pa_ps", bufs=4, space="PSUM") as paps,
                tc.tile_pool(name="pa_eps", bufs=2, space="PSUM") as paeps,
            ):
                for w0 in range(0, c.nw, CH):
                    do_er = w0 < c.wpc
                    er_sb = (pag.tile([P, CH, 2 * c.heads], F32, name="er_sb",
                                      tag="ersb") if do_er else None)
                    for si, s in enumerate(stacks):
                        xw = pax.tile([c.in_dim, CH, P], F32, name="xw", tag="x")
                        nc.sync.dma_start(
                            xw[:], xT[s][w0 : w0 + CH].rearrange("w d p -> d w p"))
                        g_sb = pag.tile([P, CH, HF], F32, name="g_sb", tag="gsb")
                        for k in range(CH):
                            g_ps = paps.tile([P, HF + c.heads], F32,
                                             name="g_ps", tag="g")
                            nc.tensor.matmul(g_ps[:], xw[:, k, :],
                                             ct[("Wg", s)][:],
                                             start=True, stop=True)
                            if do_er:
                                nc.vector.tensor_copy(
                                    er_sb[:, k, si * c.heads : (si + 1) * c.heads],
                                    g_ps[:, HF : HF + c.heads])
                            nc.any.tensor_copy(g_sb[:, k, :], g_ps[:, 0:HF])
                        nc.sync.dma_start(
                            gtab[s][w0 * P : (w0 + CH) * P, :]
                            .rearrange("(w p) d -> p w d", p=P), g_sb[:])
                    if do_er:
                        nc.sync.dma_start(
                            ertab[w0 * P : (w0 + CH) * P, :]
                            .rearrange("(w p) d -> p w d", p=P), er_sb[:])

            # ---------------- phase B: windows ----------------
            with (
                tc.tile_pool(name="pb_meta", bufs=2) as pbm,
                tc.tile_pool(name="pb_z", bufs=2) as pbz,
                tc.tile_pool(name="pb_s", bufs=2) as pbs,
                tc.tile_pool(name="pb_small", bufs=2) as pbsm,
                tc.tile_pool(name="pb_epi", bufs=2) as pbepi,
                tc.tile_pool(name="ps_acc", bufs=1, space="PSUM") as psacc,
                tc.tile_pool(name="ps_er", bufs=2, space="PSUM") as pser,
                tc.tile_pool(name="ps_bc", bufs=1, space="PSUM") as psbc,
                tc.tile_pool(name="ps_epi", bufs=2, space="PSUM") as psepi,
            ):
                nidx_reg = {} if debug_level >= 2 else None

                def _reg(n):
                    if n not in nidx_reg:
                        nidx_reg[n] = nc.gpsimd.to_reg(n)
                    return nidx_reg[n]
                for wl in range(c.wpc if debug_level >= 1 else 0):
                    idx_t = {}
                    for nm, hd, tt in [("A", idxA_d, TA), ("B", idxB_d, TB)]:
                        it = pbm.tile([P, tt * 8], I16, name=f"idx{nm}", tag=f"idx{nm}")
                        nc.sync.dma_start(it[:], hd[wl])
                        idx_t[nm] = it
                    dmb = pbm.tile([P, T], F32, name="dmb", tag="dmb")
                    nc.sync.dma_start(dmb[:], dstmb_d[wl])
                    drow = pbm.tile([1, T * P], F32, name="drow", tag="drow")
                    nc.sync.dma_start(drow[:], dstrow_d[wl])
                    wt = {}
                    for s in stacks:
                        wt[s] = pbm.tile([P, T], F32, name=f"wt{s}", tag=f"wt{s}")
                        nc.sync.dma_start(wt[s][:], wtab[s][wl])
                    erw = pbm.tile([P, 2 * c.heads], F32, name="erw", tag="erw")
                    nc.sync.dma_start(erw[:], ertab[wl * P : (wl + 1) * P, :])

                    # gathers
                    Z = {}
                    for s in stacks:
                        for gn, tt, base in [("A", TA, 0), ("B", TB, c.split)]:
                            zt = pbz.tile([P, tt, HF], F32, name=f"z{gn}{s}", tag=f"z{gn}{s}")
                            src_ap = (gtab[s][0 : c.split, :] if gn == "A"
                                      else gtab[s][c.split : c.n_pad, :])
                            if debug_level >= 2:
                                gb = c.gbatch or tt
                                for off in range(0, tt, gb):
                                    nb = min(gb, tt - off)
                                    nc.gpsimd.dma_gather(
                                        out_ap=zt[:, off : off + nb, :],
                                        in_ap=src_ap,
                                        idxs_ap=idx_t[gn][:, off * 8 : (off + nb) * 8],
                                        num_idxs=nb * P, num_idxs_reg=_reg(nb * P),
                                        elem_size=HF)
                            else:
                                nc.vector.memset(zt[:], 0.01)
                            Z[(s, gn)] = zt

                    # S one-hots (edges x nodes), per group
                    SDT = mybir.dt.bfloat16 if c.bf16 else F32
                    S = {}
                    for gn, tt, off in [("A", TA, 0), ("B", TB, TA)]:
                        st_ = pbs.tile([P, tt, P], SDT, name=f"S{gn}", tag=f"S{gn}")
                        nc.vector.tensor_tensor(
                            out=st_[:],
                            in0=ct["iotab"][:].unsqueeze(1).broadcast_to([P, tt, P]),
                            in1=dmb[:, off : off + tt].unsqueeze(2)
                                .broadcast_to([P, tt, P]),
                            op=mybir.AluOpType.is_equal)
                        S[gn] = st_

                    # ST (nodes x edges) via K=1 bcast matmul + compare
                    ST = {}
                    for gn, tt, off in [("A", TA, 0), ("B", TB, TA)]:
                        stt = pbs.tile([P, tt * P], F32, name=f"ST{gn}", tag=f"ST{gn}")
                        ncols_total = tt * P
                        ch0 = 0
                        while ch0 < ncols_total:
                            cw = min(512, ncols_total - ch0)
                            bc = psbc.tile([P, 512], F32, name="bc", tag="bc")
                            nc.tensor.matmul(
                                bc[:, 0:cw], ct["ones"][:],
                                drow[:, off * P + ch0 : off * P + ch0 + cw],
                                start=True, stop=True)
                            nc.vector.tensor_tensor(
                                out=stt[:, ch0 : ch0 + cw], in0=bc[:, 0:cw],
                                in1=ct["iotac"][:].broadcast_to([P, cw]),
                                op=mybir.AluOpType.is_equal)
                            ch0 += cw
                        ST[gn] = stt

                    # er per edge: er_ps[:, t, :] = ST_t.T @ er_win  (edges x 8)
                    er_ps = pser.tile([P, T, 2 * c.heads], F32, name="er_ps", tag="erps")
                    for t in range(T):
                        gn, tl = ("A", t) if t < TA else ("B", t - TA)
                        nc.tensor.matmul(
                            er_ps[:, t, :],
                            ST[gn][:, tl * P : (tl + 1) * P], erw[:],
                            start=True, stop=True)

                    # p = exp(leakyrelu(el + er)); c = p * w; Z *= c (per head)
                    pboth = {}
                    for gn, tt, off in [("A", TA, 0), ("B", TB, TA)]:
                        tb = pbsm.tile([P, tt, 2 * c.heads], F32, name=f"t{gn}", tag=f"t{gn}")
                        for si, s in enumerate(stacks):
                            el = (Z[(s, gn)][:]
                                  .rearrange("p t (h f) -> p t h f", h=c.heads)
                                  [:, :, :, 0:1].squeeze(3))
                            nc.vector.tensor_tensor(
                                out=tb[:, :, si * c.heads : (si + 1) * c.heads],
                                in0=el,
                                in1=er_ps[:, off : off + tt,
                                          si * c.heads : (si + 1) * c.heads],
                                op=mybir.AluOpType.add)
                        nc.vector.scalar_tensor_tensor(
                            out=tb[:], in0=tb[:], scalar=NEG, in1=tb[:],
                            op0=mybir.AluOpType.mult, op1=mybir.AluOpType.max)
                        pb_ = pbsm.tile([P, tt, 2 * c.heads], SDT, name=f"p{gn}", tag=f"p{gn}")
                        nc.scalar.activation(pb_[:], tb[:],
                                             mybir.ActivationFunctionType.Exp)
                        cb = pbsm.tile([P, tt, 2 * c.heads], F32, name=f"c{gn}", tag=f"c{gn}")
                        for si, s in enumerate(stacks):
                            nc.vector.tensor_tensor(
                                out=cb[:, :, si * c.heads : (si + 1) * c.heads],
                                in0=pb_[:, :, si * c.heads : (si + 1) * c.heads],
                                in1=wt[s][:, off : off + tt].unsqueeze(2)
                                    .broadcast_to([P, tt, c.heads]),
                                op=mybir.AluOpType.mult)
                            if c.bf16:
                                zb = pbz.tile([P, tt, HF], SDT,
                                              name=f"zb{gn}{s}", tag=f"zb{gn}{s}")
                                zdst = zb
                            else:
                                zdst = Z[(s, gn)]
                            nc.vector.tensor_tensor(
                                out=zdst[:].rearrange(
                                    "p t (h f) -> p t h f", h=c.heads),
                                in0=Z[(s, gn)][:].rearrange(
                                    "p t (h f) -> p t h f", h=c.heads),
                                in1=cb[:, :, si * c.heads : (si + 1) * c.heads]
                                    .unsqueeze(3)
                                    .broadcast_to([P, tt, c.heads, c.fdim]),
                                op=mybir.AluOpType.mult)
                            if c.bf16:
                                Z[(s, gn)] = zb
                        pboth[gn] = pb_

                    # scatter
                    num_ps = {s: psacc.tile([HF, P], F32, name=f"num{s}", tag=f"num{s}")
                              for s in stacks}
                    den_ps = psacc.tile([2 * c.heads, P], F32, name="den_ps", tag="den")
                    for t in range(T):
                        gn, tl = ("A", t) if t < TA else ("B", t - TA)
                        for s in stacks:
                            nc.tensor.matmul(
                                num_ps[s][:], Z[(s, gn)][:, tl, :],
                                S[gn][:, tl, :],
                                start=(t == 0), stop=(t == T - 1))
                        nc.tensor.matmul(
                            den_ps[:], pboth[gn][:, tl, :], S[gn][:, tl, :],
                            start=(t == 0), stop=(t == T - 1))

                    # epilogue
                    denm = pbepi.tile([2 * c.heads, P], F32, name="denm", tag="denm")
                    nc.vector.tensor_scalar(
                        out=denm[:], in0=den_ps[:], scalar1=1e-9, scalar2=None,
                        op0=mybir.AluOpType.max)
                    rec = pbepi.tile([2 * c.heads, P], F32, name="rec", tag="rec")
                    nc.vector.reciprocal(rec[:], denm[:])
                    for s in stacks:
                        dex = psepi.tile([P, P], F32, name="dex", tag="epi")
                        nc.tensor.matmul(dex[:], ct[("H2", s)][:], rec[:],
                                         start=True, stop=True)
                        dex_sb = pbepi.tile([P, P], F32, name="dex_sb", tag="dex_sb")
                        nc.any.tensor_copy(dex_sb[:], dex[:])
                        sca = pbepi.tile([HF, P], F32, name="sca", tag="sca")
                        nc.vector.tensor_tensor(out=sca[:], in0=num_ps[s][:],
                                                in1=dex_sb[:],
                                                op=mybir.AluOpType.mult)
                        hps = psepi.tile([OC, P], F32, name="hps", tag="epi")
                        nc.tensor.matmul(hps[:], ct[("Tinv", s)][:], sca[:],
                                         start=True, stop=True)
                        hsb = pbepi.tile([OC, P], F32, name="hsb", tag="hsb")
                        nc.scalar.activation(
                            hsb[:], hps[:],
                            (mybir.ActivationFunctionType.Identity if last
                             else mybir.ActivationFunctionType.Relu),
                            bias=ct[("bcol", s)][:], scale=1.0)
                        nc.sync.dma_start(out_t[s][wl], hsb[:])

    return _finish(nc)


def _finish(nc):
    nc.compile()
    return nc


# ------------------------------------------------------------ full pipeline

def make_in_maps(cfg, g, cc, xT_am_full, xT_ph_full, w_am_tab, w_ph_tab,
                 conv_am, conv_ph):
    """Build per-core input dicts for one layer launch."""
    maps = []
    for core in range(cfg.n_cores):
        m = dict(
            xT_am=rotate_tiles(xT_am_full, core, cfg),
            xT_ph=rotate_tiles(xT_ph_full, core, cfg),
            Wg_am=conv_am["Wg"], Tinv_am=conv_am["Tinv"],
            bcol_am=conv_am["bcol"], Wg_ph=conv_ph["Wg"],
            Tinv_ph=conv_ph["Tinv"], bcol_ph=conv_ph["bcol"],
            H2_am=cc["H2am"], H2_ph=cc["H2ph"], iotab=cc["iotab"],
            iotac=cc["iotac"], ones1=cc["ones1"],
            idxA=g["idxA"][core], idxB=g["idxB"][core],
            dstmb=g["dstmb"][core], dstrow=g["dstrow"][core],
            wtab_am=w_am_tab[core], wtab_ph=w_ph_tab[core],
        )
        maps.append(m)
    return maps


def assemble(outs, cfg, oc):
    """per-core out (wpc, OC, P) list -> (nw*P, OC) permuted-node-major."""
    full = np.concatenate([o.reshape(cfg.wpc, oc, P) for o in outs], 0)
    return full.transpose(0, 2, 1).reshape(cfg.n_pad, oc)


def run_pipeline(inputs, cfg, runner):
    """runner(nc, in_maps) -> list of per-core {name: np.ndarray} outputs."""
    g = prep_graph(np.asarray(inputs["src"]), np.asarray(inputs["dst"]), cfg)
    cc = consts_np(cfg)
    w_am = edge_w_tables(g, inputs["am_exist"], cfg)
    w_ph = edge_w_tables(g, inputs["exist"], cfg)

    conv0a = prep_conv(inputs["W0a"], inputs["al0a"], inputs["ar0a"],
                       inputs["b0a"], False, cfg)
    conv0p = prep_conv(inputs["W0p"], inputs["al0p"], inputs["ar0p"],
                       inputs["b0p"], False, cfg)
    conv1a = prep_conv(inputs["W1a"], inputs["al1a"], inputs["ar1a"],
                       inputs["b1a"], True, cfg)
    conv1p = prep_conv(inputs["W1p"], inputs["al1p"], inputs["ar1p"],
                       inputs["b1p"], True, cfg)

    xT_am = to_xT_tiled(np.asarray(inputs["x_am"]), g, cfg)
    xT_ph = to_xT_tiled(np.asarray(inputs["x_ph"]), g, cfg)

    nc0 = build_layer(cfg, g["TA"], g["TB"], last=False)
    maps0 = make_in_maps(cfg, g, cc, xT_am, xT_ph, w_am, w_ph, conv0a, conv0p)
    outs0 = runner(nc0, maps0)

    hT_am = np.concatenate([o["out_am"] for o in outs0], 0)  # (nw,128,128)
    hT_ph = np.concatenate([o["out_ph"] for o in outs0], 0)

    nc1 = build_layer(cfg, g["TA"], g["TB"], last=True)
    maps1 = make_in_maps(cfg, g, cc, hT_am, hT_ph, w_am, w_ph, conv1a, conv1p)
    outs1 = runner(nc1, maps1)

    oam = assemble([o["out_am"] for o in outs1], cfg, cfg.fdim)
    oph = assemble([o["out_ph"] for o in outs1], cfg, cfg.fdim)
    res_am = np.zeros((cfg.n_nodes, cfg.fdim), np.float32)
    res_ph = np.zeros((cfg.n_nodes, cfg.fdim), np.float32)
    nid = g["new_id"][: cfg.n_nodes]
    res_am[:] = oam[nid]
    res_ph[:] = oph[nid]
    return res_am, res_ph


# ------------------------------------------------------------ timed runner

def run_layer_timed(nc, in_maps, n_cores, repeats=3):
    """Execute with device-resident inputs; returns (results, times)."""
    import time as _time
    import jax
    from jax.sharding import Mesh, PartitionSpec, NamedSharding
    from jax.experimental.shard_map import shard_map
    from concourse import bass2jax

    bass2jax.install_neuronx_cc_hook()
    part_name = (nc.partition_id_tensor.name
                 if nc.partition_id_tensor is not None else None)
    in_names, out_names, out_avals, zero_outs = [], [], [], []
    for alloc in nc.m.functions[0].allocations:
        if not isinstance(alloc, mybir.MemoryLocationSet):
            continue
        name = alloc.memorylocations[0].name
        if alloc.kind == "ExternalInput":
            if name != part_name:
                in_names.append(name)
        elif alloc.kind == "ExternalOutput":
            out_names.append(name)
            shape = tuple(alloc.tensor_shape)
            dtype = mybir.dt.np(alloc.dtype)
            out_avals.append(jax.core.ShapedArray(shape, dtype))
            zero_outs.append(np.zeros(shape, dtype))
    n_params = len(in_names)
    all_in = list(in_names + out_names)
    if part_name is not None:
        all_in.append(part_name)

    def _body(*args):
        operands = list(args)
        if part_name is not None:
            operands.append(bass2jax.partition_id_tensor())
        outs = bass2jax._bass_exec_p.bind(
            *operands, out_avals=tuple(out_avals), in_names=tuple(all_in),
            out_names=tuple(out_names), lowering_input_output_aliases=(),
            sim_require_finite=True, sim_require_nnan=True, nc=nc)
        return tuple(outs)

    devices = jax.devices()[:n_cores]
    mesh = Mesh(np.asarray(devices), ("core",))
    spec = PartitionSpec("core")
    nin = n_params + len(out_names)
    f = jax.jit(shard_map(_body, mesh=mesh, in_specs=(spec,) * nin,
                          out_specs=(spec,) * len(out_names), check_rep=False))
    concat_in = [np.concatenate([np.asarray(m[nm]) for m in in_maps], 0)
                 for nm in in_names]
    concat_zeros = [np.zeros((n_cores * z.shape[0], *z.shape[1:]), z.dtype)
                    for z in zero_outs]
    sh = NamedSharding(mesh, spec)
    dev_in = [jax.device_put(a, sh) for a in concat_in]
    dev_zero = [jax.device_put(a, sh) for a in concat_zeros]
    outs = f(*dev_in, *dev_zero)
    jax.block_until_ready(outs)
    ts = []
    for _ in range(repeats):
        t0 = _time.perf_counter()
        o2 = f(*dev_in, *dev_zero)
        jax.block_until_ready(o2)
        ts.append(_time.perf_counter() - t0)
    res = []
    for c in range(n_cores):
        res.append({nm: np.asarray(outs[i]).reshape(n_cores, *out_avals[i].shape)[c]
                    for i, nm in enumerate(out_names)})
    return res, ts


def baseline_overhead(n_cores, repeats=5):
    """Dispatch+network floor: trivial 8-core kernel timed the same way."""
    nc = bacc.Bacc("TRN2", target_bir_lowering=False, debug=False)
    x = nc.dram_tensor("x", [P, P], F32, kind="ExternalInput")
    y = nc.dram_tensor("y", [P, P], F32, kind="ExternalOutput")
    with tile.TileContext(nc) as tc:
        with tc.tile_pool(name="p", bufs=1) as p:
            t = p.tile([P, P], F32)
            nc.sync.dma_start(t[:], x[:])
            nc.scalar.mul(t[:], t[:], 2.0)
            nc.sync.dma_start(y[:], t[:])
    nc.compile()
    maps = [{"x": np.zeros((P, P), np.float32)} for _ in range(n_cores)]
    _, ts = run_layer_timed(nc, maps, n_cores, repeats=repeats)
    return min(ts)


# ------------------------------------------------------------ kernel entry

_PERF = {"exec_ns": 0.0, "launch_info": []}


def _hw_runner(cfg, measure):
    from concourse.bass_utils import run_bass_kernel_spmd

    def run(nc, in_maps):
        if measure:
            res, ts = run_layer_timed(nc, in_maps, cfg.n_cores, repeats=3)
            _PERF["launch_info"].append(min(ts))
            return [{k: r[k].reshape(cfg.wpc, -1, P)
                     for k in ("out_am", "out_ph")} for r in res]
        res = run_bass_kernel_spmd(nc, in_maps,
                                   core_ids=list(range(cfg.n_cores)))
        return [{k: r[k].reshape(cfg.wpc, -1, P)
                 for k in ("out_am", "out_ph")} for r in res.results]
    return run


def kernel(**inputs):
    """Full DUPLEX-GAT forward on 8 trn2 cores. Returns (h_am, h_ph)."""
    import os
    cfg = Cfg(gbatch=int(os.environ.get("GAT_GBATCH", "4")),
              bf16=bool(int(os.environ.get("GAT_BF16", "0"))))
    measure = bool(int(os.environ.get("GAT_MEASURE", "0")))
    res_am, res_ph = run_pipeline(inputs, cfg, _hw_runner(cfg, measure))
    return res_am, res_ph



# revision 29
# speedup vs baseline: 17.6065x; 17.6065x over previous
"""DUPLEX GAT on trn2 — kernel builder + host glue (v2, bf16).

Design:
  - Nodes permuted into NW windows of 128 (degree-balanced), padded to N_pad.
  - Per core c the node tables are ROTATED so that core-local dst windows are
    rows [0, wpc*128) of its private tables -> one SPMD program, all per-core
    variation lives in input data.
  - Phase A (both stacks fused): g rows [g_am(128)|g_ph(128)] bf16 in DRAM
    (per-head basis T_bd puts el in g col h*F); er table rows
    [er_am(4)|er_ph(4)] bf16 for own windows.
  - Phase B per dst-window: one packed meta DMA (gather idx A/B/dst, ln(w)
    fp32, dst-mod bf16); two Z-gathers pull BOTH stacks (interleaved g rows);
    one er-gather by dst replaces the old ST/one-hot broadcast entirely;
    S one-hot via iota compare (bf16); p = exp(lrelu(el+er)) into the RHS
    den cols; cbe = exp(lrelu+lnw) broadcast over F on the Act engine;
    Z *= cbe (packed bf16, 2x DVE); ONE accumulating matmul stream per tile:
    ACC[128n, 264] += S_t.T @ [Z_am|Z_ph|p] (num both stacks + den fused);
    epilogue: rec=1/den, PE transposes + H2/Tinv matmuls, bias(+ReLU).
  - Softmax max-subtraction dropped (exact cancellation; logits are O(3)).
"""
import sys
sys.path.insert(0, '/opt/trn_rl_repo')
from dataclasses import dataclass

import numpy as np
import ml_dtypes

import concourse.bass as bass
import concourse.bacc as bacc
import concourse.tile as tile
from concourse import mybir, library_config

F32 = mybir.dt.float32
BF = mybir.dt.bfloat16
I16 = mybir.dt.int16
NPBF = ml_dtypes.bfloat16
P = 128
NEG = 0.2


def _patch_drain_split():
    """The installed walrus rejects >1 sem wait on the kernel-tail Drain;
    split the waits across a chain of drains."""
    import bass_rust
    from concourse.tile import ScopedClock

    def patched(self, tick_clock, wait_clock):
        nc = self.nc
        drain_inst = nc.sync.drain()
        wait_clock.add_sem_waits(
            drain_inst.ins, ScopedClock({None: tick_clock.global_clock}))
        si = drain_inst.ins.sync_info
        waits = list(si.on_wait) if si is not None else []
        if len(waits) > 1:
            si.on_wait = waits[:1]
            for i in range(1, len(waits)):
                d2 = nc.sync.drain()
                d2.ins.sync_info = bass_rust.SyncInfo(
                    on_wait=waits[i : i + 1], on_update=[])
        nc.all_engine_barrier()
        popped = nc._tile_sem_poison_stack.pop()
        assert popped is self._sem_poison
        nc.clear_and_free_semaphores(list(self.sems.allocated().values()))
        nc.all_engine_barrier()

    tile.TileContext._drain_and_barrier = patched


_patch_drain_split()


def _patch_loud_ncc():
    """Surface compile-hook exceptions (swallowed by the PJRT plugin)."""
    import traceback
    from concourse import bass2jax
    if getattr(bass2jax, "_loud_ncc", False):
        return
    bass2jax._loud_ncc = True
    orig = bass2jax.neuronx_cc_hook

    def logged(*a, **k):
        try:
            return orig(*a, **k)
        except BaseException:
            with open("/tmp/ncc_hook_err.log", "a") as f:
                f.write(traceback.format_exc() + "\n")
            raise

    bass2jax.neuronx_cc_hook = logged


_patch_loud_ncc()


@dataclass
class Cfg:
    n_nodes: int = 50000
    n_edges: int = 800000
    n_cores: int = 8
    wpc: int = 49           # windows per core
    split: int = 32768      # int16 gather split (rows per table half A)
    heads: int = 4
    fdim: int = 32
    in_dim: int = 128

    @property
    def nw(self):
        return self.n_cores * self.wpc

    @property
    def n_pad(self):
        return self.nw * P


# ----------------------------------------------------------------- host prep

def balance_windows(deg, cfg):
    """Assign nodes (incl pad) to windows, balancing total degree via LPT."""
    import heapq
    n_pad, nw = cfg.n_pad, cfg.nw
    degp = np.zeros(n_pad, np.int64)
    degp[: len(deg)] = deg
    order = np.argsort(-degp, kind="stable")
    heap = [(0, w, 0) for w in range(nw)]  # (load, window, count)
    heapq.heapify(heap)
    win_of = np.empty(n_pad, np.int32)
    slot_of = np.empty(n_pad, np.int32)
    for node in order:
        load, w, cnt = heapq.heappop(heap)
        win_of[node] = w
        slot_of[node] = cnt
        cnt += 1
        if cnt < P:
            heapq.heappush(heap, (load + degp[node], w, cnt))
    new_id = win_of.astype(np.int64) * P + slot_of
    return new_id  # (n_pad,) position of each (padded) node id


def wrap_idx(idx):
    """(n,) -> (128, n//16) int16 wrapped layout for dma_gather."""
    n = len(idx)
    blk = np.asarray(idx, np.int16).reshape(n // 16, 16).T
    return np.tile(blk, (8, 1))


def prep_graph(src, dst, cfg):
    """Build per-core packed meta tables. Returns dict."""
    c = cfg
    deg = np.bincount(dst, minlength=c.n_nodes)
    new_id = balance_windows(deg, c)              # old(+pad) -> new position
    node_at = np.full(c.n_pad, -1, np.int64)      # new position -> old id
    node_at[new_id] = np.arange(c.n_pad)

    src_n = new_id[src]
    dst_n = new_id[dst]
    order = np.argsort(dst_n, kind="stable")
    src_s, dst_s = src_n[order], dst_n[order]
    win_s = dst_s // P
    bounds = np.searchsorted(win_s, np.arange(c.nw + 1))

    # per-core rotated row of a (new-space) node position
    def rot_rows(core, pos):
        return ((pos // P - core * c.wpc) % c.nw) * P + pos % P

    TA = TB = 0
    grp_masks = {}
    for core in range(c.n_cores):
        for wl in range(c.wpc):
            g = core * c.wpc + wl
            lo, hi = bounds[g], bounds[g + 1]
            rr = rot_rows(core, src_s[lo:hi])
            mA = rr < c.split
            grp_masks[(core, wl)] = (lo, hi, rr, mA)
            nA = int(mA.sum())
            nB = int((hi - lo) - nA)
            TA = max(TA, -(-nA // P))
            TB = max(TB, -(-nB // P))
    TA = max(TA, 1)
    TB = max(TB, 1)
    T = TA + TB

    # packed int16 meta columns per window:
    #   [0, TA*8)          idxA   (group-A gather rows, wrapped)
    #   [TA*8, (TA+TB)*8)  idxB
    #   [A2, A2+T*4)       lnw    fp32 [P, T, 2] (ln edge weight, am|ph)
    #   [A3, A3+T*2)       dmb    fp32 [P, T]    (dst mod 128, pad -1000)
    #   [A4, A4+T*128)     drow_b bf16 [P, T*P]  (dst mod, row-bcast to all
    #                                             partitions, t-major)
    A2 = (TA + TB) * 8
    A3 = A2 + T * 4
    A4 = A3 + T * 2
    M = A4 + T * P  # even: fp32 bitcast needs even strides

    nco, wpc = c.n_cores, c.wpc
    meta = np.zeros((nco, wpc, P, M), np.int16)
    epos = np.full((nco, wpc, T * P), -1, np.int64)  # orig edge slot (sorted)

    for core in range(nco):
        for wl in range(wpc):
            lo, hi, rr, mA = grp_masks[(core, wl)]
            iA = np.where(mA)[0]
            iB = np.where(~mA)[0]
            la = np.zeros(TA * P, np.int64)   # gather rows group A (pad->0)
            lb = np.zeros(TB * P, np.int64)
            la[: len(iA)] = rr[iA]
            lb[: len(iB)] = rr[iB] - c.split
            m = meta[core, wl]
            m[:, 0 : TA * 8] = wrap_idx(la)
            m[:, TA * 8 : A2] = wrap_idx(lb)
            dmb = np.full(T * P, -1000.0, np.float32)
            dmb[: len(iA)] = (dst_s[lo:hi][iA] % P).astype(np.float32)
            dmb[TA * P : TA * P + len(iB)] = (dst_s[lo:hi][iB] % P).astype(np.float32)
            m[:, A3 : A3 + T * 2] = (
                np.ascontiguousarray(dmb.reshape(T, P).T).view(np.int16))
            m[:, A4 : A4 + T * P] = (
                dmb.astype(NPBF).view(np.int16)[None, :])
            ep = np.full(T * P, -1, np.int64)
            ep[: len(iA)] = lo + iA
            ep[TA * P : TA * P + len(iB)] = lo + iB
            epos[core, wl] = ep

    return dict(
        new_id=new_id, node_at=node_at, order=order, TA=TA, TB=TB,
        meta=meta, epos=epos, A2=A2, A3=A3, A4=A4, M=M,
    )


def fill_lnw(g, w_am, w_ph, cfg):
    """Write ln(edge weight) fp32 [P, T, 2] into the packed meta tables."""
    c = cfg
    T = g["TA"] + g["TB"]
    A2, A3 = g["A2"], g["A3"]
    ws = np.stack([np.asarray(w_am), np.asarray(w_ph)], 1)[g["order"]]  # (E, 2)
    lnws = np.log(np.maximum(ws, 1e-38)).astype(np.float32)
    ep = g["epos"]  # (nco, wpc, T*P)
    val = np.where(ep[..., None] >= 0,
                   lnws[np.clip(ep, 0, None)], 0.0).astype(np.float32)
    # (nco, wpc, T*P, 2) -> per window [P, T, 2]
    nco, wpc = c.n_cores, c.wpc
    v = val.reshape(nco, wpc, T, P, 2).transpose(0, 1, 3, 2, 4)
    flat = np.ascontiguousarray(v).view(np.int16).reshape(nco, wpc, P, T * 4)
    g["meta"][:, :, :, A2:A3] = flat


def prep_conv(W, al, ar, b, mean_heads, cfg):
    """Derived weights for one conv. Returns dict of arrays."""
    H, F = cfg.heads, cfg.fdim
    W = np.asarray(W, np.float64)
    al = np.asarray(al, np.float64)
    ar = np.asarray(ar, np.float64)
    b = np.asarray(b, np.float64)
    T_bd = np.zeros((H * F, H * F))
    for h in range(H):
        a = al[h]
        Mx = np.concatenate([a[:, None], np.eye(F)[:, : F - 1]], 1)
        Q, _ = np.linalg.qr(Mx)
        blk = np.concatenate([a[:, None], Q[:, 1:]], 1)
        T_bd[h * F : (h + 1) * F, h * F : (h + 1) * F] = blk
    Tinv = np.linalg.inv(T_bd)
    R_bd = np.zeros((H * F, H))
    for h in range(H):
        R_bd[h * F : (h + 1) * F, h] = ar[h]
    out = dict(
        Wg=np.concatenate([W @ T_bd, W @ R_bd], 1).astype(NPBF),
    )
    if mean_heads:
        Mm = np.zeros((H * F, F))
        for h in range(H):
            Mm[h * F : (h + 1) * F] = np.eye(F) / H
        out["Tinv"] = (Tinv @ Mm).astype(np.float32)          # (128, 32)
        out["bcol"] = (b.reshape(H, F).mean(0))[:, None].astype(np.float32)
    else:
        out["Tinv"] = Tinv.astype(np.float32)                  # (128, 128)
        out["bcol"] = b[:, None].astype(np.float32)            # (128, 1)
    return out


def consts_np(cfg):
    H, F = cfg.heads, cfg.fdim
    iotab = np.tile(np.arange(P, dtype=np.float32)[None, :], (P, 1)).astype(NPBF)
    iotac = np.arange(P, dtype=np.float32)[:, None]
    ident = np.eye(P, dtype=np.float32)
    H2 = np.zeros((2, 2 * H, H * F), np.float32)  # [stack, 8, 128]
    for s in range(2):
        for h in range(H):
            H2[s, s * H + h, h * F : (h + 1) * F] = 1.0
    return dict(iotab=iotab, iotac=iotac, ident=ident,
                H2am=H2[0], H2ph=H2[1])


def rotate_tiles(x_tiled, core, cfg):
    """x_tiled (D, nw, 128) -> rotated copy for `core` (roll window axis)."""
    rot = np.roll(np.arange(cfg.nw), -core * cfg.wpc)
    return np.ascontiguousarray(x_tiled[:, rot])


def to_xT_tiled(x, g, cfg):
    """x (n_nodes, D) -> permuted d-major tiles (D, nw, 128) bf16."""
    n_pad = cfg.n_pad
    D = x.shape[1]
    xp = np.zeros((n_pad, D), np.float32)
    real = g["node_at"] >= 0
    idx = g["node_at"][real]
    keep = idx < cfg.n_nodes
    xp[np.where(real)[0][keep]] = np.asarray(x, np.float32)[idx[keep]]
    return np.ascontiguousarray(
        xp.reshape(cfg.nw, P, D).transpose(2, 0, 1)).astype(NPBF)


# ------------------------------------------------------------ layer program

def build_layer(cfg, TA, TB, last, debug_level=3):
    """One Bass program: both stacks of one GAT layer. Returns nc."""
    c = cfg
    T = TA + TB
    HF = c.heads * c.fdim            # 128
    H2n = 2 * c.heads                # 8
    OC = c.fdim if last else HF      # output channels per node
    A2 = (TA + TB) * 8
    A3 = A2 + T * 4
    A4 = A3 + T * 2
    M = A4 + T * P
    nc = bacc.Bacc("TRN2", target_bir_lowering=False, debug=False)

    inp = {}

    def dram_in(name, shape, dt=F32):
        inp[name] = nc.dram_tensor(name, list(shape), dt, kind="ExternalInput")
        return inp[name]

    stacks = ("am", "ph")
    xT = {s: dram_in(f"xT_{s}", (c.in_dim, c.nw, P), BF) for s in stacks}
    Wg = {s: dram_in(f"Wg_{s}", (c.in_dim, HF + c.heads), BF) for s in stacks}
    Tinv = {s: dram_in(f"Tinv_{s}", (HF, OC)) for s in stacks}
    bcol = {s: dram_in(f"bcol_{s}", (OC, 1)) for s in stacks}
    H2 = {s: dram_in(f"H2_{s}", (H2n, HF)) for s in stacks}
    iotab_d = dram_in("iotab", (P, P), BF)
    iotac_d = dram_in("iotac", (P, 1))
    ident_d = dram_in("ident", (P, P))
    meta_d = dram_in("meta", (c.wpc, P, M), I16)

    out_dt = F32 if last else BF
    out_d = nc.dram_tensor("out", [c.wpc, OC, 2, P], out_dt,
                           kind="ExternalOutput")

    with tile.TileContext(nc) as tc:
        with (
            tc.tile_pool(name="dram", bufs=1, space="DRAM") as dpool,
            tc.tile_pool(name="const", bufs=1) as cpool,
        ):
            gtab = dpool.tile([c.n_pad, 2 * HF], BF, name="gtab", tag="gtab")
            ertab = dpool.tile([c.wpc * P, H2n], BF, name="ertab", tag="ertab")

            ct = {}
            for nm, hd in [("iotab", iotab_d), ("iotac", iotac_d),
                           ("ident", ident_d)]:
                t = cpool.tile(list(hd.shape), hd.dtype, name=f"ct_{nm}")
                nc.sync.dma_start(t[:], hd[:])
                ct[nm] = t
            for s in stacks:
                for nm, hd in [("Wg", Wg[s]), ("Tinv", Tinv[s]),
                               ("bcol", bcol[s]), ("H2", H2[s])]:
                    t = cpool.tile(list(hd.shape), hd.dtype, name=f"ct_{nm}_{s}")
                    nc.scalar.dma_start(t[:], hd[:])
                    ct[(nm, s)] = t

            # ---------------- phase A: g / er tables ----------------
            CH = 14
            with (
                tc.tile_pool(name="pa_x", bufs=3) as pax,
                tc.tile_pool(name="pa_g", bufs=3) as pag,
                tc.tile_pool(name="pa_ps", bufs=4, space="PSUM") as paps,
            ):
                for w0 in range(0, c.nw, CH):
                    ner = max(0, min(CH, c.wpc - w0))
                    xw = {}
                    for si, s in enumerate(stacks):
                        xw[s] = pax.tile([c.in_dim, CH, P], BF,
                                         name="xw", tag=f"x{s}")
                        eng = nc.sync if si == 0 else nc.scalar
                        eng.dma_start(xw[s][:], xT[s][:, w0 : w0 + CH])
                    g_cat = pag.tile([P, CH, 2, HF], BF, name="g_cat", tag="gcat")
                    er_sb = (pag.tile([P, CH, H2n], BF, name="er_sb",
                                      tag="ersb") if ner else None)
                    for k in range(CH):
                        g_ps = paps.tile([P, 2, HF + c.heads], F32,
                                         name="g_ps", tag="g")
                        for si, s in enumerate(stacks):
                            nc.tensor.matmul(g_ps[:, si, :], xw[s][:, k, :],
                                             ct[("Wg", s)][:],
                                             start=True, stop=True)
                        if k % 2 == 0:
                            nc.vector.tensor_copy(g_cat[:, k, :, :],
                                                  g_ps[:, :, 0:HF])
                        else:
                            nc.scalar.activation(
                                g_cat[:, k, :, :], g_ps[:, :, 0:HF],
                                mybir.ActivationFunctionType.Identity)
                        if k < ner:
                            nc.vector.tensor_copy(
                                er_sb[:, k, :].rearrange(
                                    "p (s h) -> p s h", s=2),
                                g_ps[:, :, HF : HF + c.heads])
                    nc.scalar.dma_start(
                        gtab[w0 * P : (w0 + CH) * P, :]
                        .rearrange("(w p) d -> p w d", p=P),
                        g_cat[:].rearrange("p w s d -> p w (s d)"))
                    if ner:
                        nc.sync.dma_start(
                            ertab[w0 * P : (w0 + ner) * P, :]
                            .rearrange("(w p) d -> p w d", p=P),
                            er_sb[:, 0:ner, :])

            # ---------------- phase B: windows ----------------
            with (
                tc.tile_pool(name="pb_meta", bufs=6) as pbm,
                tc.tile_pool(name="pb_rhs", bufs=4) as pbr,
                tc.tile_pool(name="pb_s", bufs=3) as pbs,
                tc.tile_pool(name="pb_sm", bufs=4) as pbsm,
                tc.tile_pool(name="pb_epi", bufs=3) as pbe,
                tc.tile_pool(name="ps_acc", bufs=3, space="PSUM") as psacc,
                tc.tile_pool(name="ps_epi", bufs=4, space="PSUM") as psepi,
            ):
                nidx_reg = {}

                def _reg(n):
                    if n not in nidx_reg:
                        nidx_reg[n] = nc.gpsimd.to_reg(n)
                    return nidx_reg[n]

                meta_tiles = {}

                def load_meta(w):
                    mt = pbm.tile([P, M], I16, name="meta", tag="meta")
                    nc.sync.dma_start(mt[:], meta_d[w])
                    meta_tiles[w] = mt

                PRE = 4
                nwin = c.wpc if debug_level >= 1 else 0
                for w in range(min(PRE, nwin)):
                    load_meta(w)
                for wl in range(nwin):
                    if wl + PRE < nwin:
                        load_meta(wl + PRE)
                    meta = meta_tiles.pop(wl)
                    idx = {"A": meta[:, 0 : TA * 8], "B": meta[:, TA * 8 : A2]}
                    lnw = (meta[:, A2:A3].bitcast(F32)
                           .rearrange("p (t s) -> p t s", s=2))
                    dmb = meta[:, A3:A4].bitcast(F32)
                    drow_b = meta[:, A4 : A4 + T * P].bitcast(BF)
                    erw = pbsm.tile([P, H2n], BF, name="erw", tag="erw")
                    nc.sync.dma_start(
                        erw[:], ertab[wl * P : (wl + 1) * P, :]
                        .rearrange("(w p) d -> p (w d)", p=P))

                    # gathers: both stacks per group + er by dst
                    rhs = {}
                    ptile = {}
                    for gn, tt, lo, hi in (("A", TA, 0, c.split),
                                           ("B", TB, c.split, c.n_pad)):
                        rt = pbr.tile([P, tt, 2 * HF], BF, name=f"rhs{gn}",
                                      tag=f"rhs{gn}")
                        if debug_level >= 2:
                            gb = 4  # ring-safe gather chunk (tiles/call)
                            for off in range(0, tt, gb):
                                nb = min(gb, tt - off)
                                nc.gpsimd.dma_gather(
                                    out_ap=rt[:, off : off + nb, :],
                                    in_ap=gtab[lo:hi, :],
                                    idxs_ap=idx[gn][:, off * 8 : (off + nb) * 8],
                                    num_idxs=nb * P,
                                    num_idxs_reg=_reg(nb * P),
                                    elem_size=2 * HF)
                        else:
                            nc.vector.memset(rt[:], 0.01)
                        rhs[gn] = rt
                        ptile[gn] = pbr.tile([P, tt, H2n], BF, name=f"p{gn}",
                                             tag=f"p{gn}")
                    # ST one-hot (nodes x edges) from host-bcast drow
                    ST = pbs.tile([P, T * P], BF, name="ST", tag="ST")
                    nc.vector.tensor_scalar(
                        out=ST[:], in0=drow_b, scalar1=ct["iotac"][:],
                        scalar2=None, op0=mybir.AluOpType.is_equal)

                    # scatter acc + er broadcast share one PSUM bank:
                    # cols [0,256) num, [256,264) den, [264,264+T*8) er/edge
                    ACCB = psacc.tile([P, 2 * HF + H2n + T * H2n], F32,
                                      name="acc", tag="acc")
                    ACC = ACCB[:, 0 : 2 * HF]
                    EB = 2 * HF + H2n
                    er_e = (ACCB[:, EB : EB + T * H2n]
                            .rearrange("p (t h) -> p t h", h=H2n))
                    # one accumulation group spans the whole bank:
                    # er t=0 start=True zeroes it; den t=T-1 stop=True ends
                    for t in range(T):
                        nc.tensor.matmul(
                            ACCB[:, EB + t * H2n : EB + (t + 1) * H2n],
                            ST[:, t * P : (t + 1) * P], erw[:],
                            start=(t == 0), stop=False,
                            skip_group_check=True)

                    # S one-hots (edges x nodes), per tile (4x DVE mode)
                    S = {}
                    for gn, tt, off in (("A", TA, 0), ("B", TB, TA)):
                        st_ = pbs.tile([P, tt, P], BF, name=f"S{gn}",
                                       tag=f"S{gn}")
                        for tl in range(tt):
                            nc.vector.tensor_scalar(
                                out=st_[:, tl, :], in0=ct["iotab"][:],
                                scalar1=dmb[:, off + tl : off + tl + 1],
                                scalar2=None,
                                op0=mybir.AluOpType.is_equal)
                        S[gn] = st_

                    # logits -> p (den cols) and cbe -> Z scale
                    for gn, tt, off in (("A", TA, 0), ("B", TB, TA)):
                        rt = rhs[gn]
                        el = (rt[:, :, 0 : 2 * HF]
                              .rearrange("p t (x f) -> p t x f", f=c.fdim)
                              [:, :, :, 0])
                        tb = pbsm.tile([P, tt, H2n], F32, name=f"tb{gn}",
                                       tag=f"tb{gn}")
                        nc.vector.tensor_tensor(
                            out=tb[:], in0=el,
                            in1=er_e[:, off : off + tt, :],
                            op=mybir.AluOpType.add)
                        nc.vector.scalar_tensor_tensor(
                            out=tb[:], in0=tb[:], scalar=NEG, in1=tb[:],
                            op0=mybir.AluOpType.mult, op1=mybir.AluOpType.max)
                        nc.scalar.activation(
                            ptile[gn][:], tb[:],
                            mybir.ActivationFunctionType.Exp)
                        tbw = pbsm.tile([P, tt, 2, c.heads], F32,
                                        name=f"tw{gn}", tag=f"tw{gn}")
                        nc.vector.tensor_tensor(
                            out=tbw[:],
                            in0=tb[:].rearrange("p t (s h) -> p t s h", s=2),
                            in1=lnw[:, off : off + tt, :].unsqueeze(3)
                                .broadcast_to([P, tt, 2, c.heads]),
                            op=mybir.AluOpType.add)
                        cbe = pbsm.tile([P, tt, 2, c.heads, c.fdim], BF,
                                        name=f"cb{gn}", tag=f"cb{gn}")
                        for si in range(2):
                            nc.scalar.activation(
                                cbe[:, :, si], tbw[:, :, si].unsqueeze(3)
                                .broadcast_to([P, tt, c.heads, c.fdim]),
                                mybir.ActivationFunctionType.Exp)
                        for si in range(2):
                            nc.vector.tensor_tensor(
                                out=rt[:, :, si * HF : (si + 1) * HF],
                                in0=rt[:, :, si * HF : (si + 1) * HF],
                                in1=cbe[:, :, si].rearrange(
                                    "p t h f -> p t (h f)"),
                                op=mybir.AluOpType.mult)

                    # scatter: fused num (both stacks) + den
                    for t in range(T):
                        gn, tl = ("A", t) if t < TA else ("B", t - TA)
                        nc.tensor.matmul(
                            ACC, S[gn][:, tl, :], rhs[gn][:, tl, :],
                            start=False, stop=False, skip_group_check=True)
                        nc.tensor.matmul(
                            ACCB[:, 2 * HF : 2 * HF + H2n],
                            S[gn][:, tl, :], ptile[gn][:, tl, :],
                            start=False, stop=(t == T - 1),
                            skip_group_check=True)

                    # epilogue
                    denm = pbe.tile([P, H2n], F32, name="denm", tag="denm")
                    nc.vector.tensor_scalar(
                        out=denm[:], in0=ACCB[:, 2 * HF : 2 * HF + H2n],
                        scalar1=1e-9, scalar2=None, op0=mybir.AluOpType.max)
                    rec = pbe.tile([P, H2n], F32, name="rec", tag="rec")
                    nc.vector.reciprocal(rec[:], denm[:])
                    recT_ps = psepi.tile([H2n, P], F32, name="recT_ps",
                                         tag="epi")
                    nc.tensor.transpose(recT_ps[:], rec[:], ct["ident"][:])
                    recT = pbe.tile([H2n, P], F32, name="recT", tag="recTs")
                    nc.vector.tensor_copy(recT[:], recT_ps[:])
                    hout = pbe.tile([OC, 2, P], out_dt, name="hout", tag="hout")
                    for si, s in enumerate(stacks):
                        numT = pbe.tile([P, HF], F32, name="numT",
                                        tag=f"numT{s}")
                        if si == 0:
                            nc.vector.tensor_copy(
                                numT[:], ACCB[:, si * HF : (si + 1) * HF])
                        else:
                            nc.scalar.activation(
                                numT[:], ACCB[:, si * HF : (si + 1) * HF],
                                mybir.ActivationFunctionType.Identity)
                        num_ps = psepi.tile([HF, P], F32, name="num_ps",
                                            tag="epi")
                        nc.tensor.transpose(num_ps[:], numT[:], ct["ident"][:])
                        dex_ps = psepi.tile([HF, P], F32, name="dex_ps",
                                            tag="epi")
                        nc.tensor.matmul(dex_ps[:], ct[("H2", s)][:], recT[:],
                                         start=True, stop=True)
                        dex_sb = pbe.tile([HF, P], F32, name="dex_sb",
                                          tag=f"dexs{s}")
                        nc.scalar.activation(
                            dex_sb[:], dex_ps[:],
                            mybir.ActivationFunctionType.Identity)
                        sca = pbe.tile([HF, P], F32, name="sca", tag=f"sca{s}")
                        nc.vector.tensor_tensor(out=sca[:], in0=num_ps[:],
                                                in1=dex_sb[:],
                                                op=mybir.AluOpType.mult)
                        hps = psepi.tile([OC, P], F32, name="hps",
                                         tag="epi")
                        nc.tensor.matmul(hps[:], ct[("Tinv", s)][:], sca[:],
                                         start=True, stop=True)
                        nc.scalar.activation(
                            hout[:, si, :], hps[:],
                            (mybir.ActivationFunctionType.Identity if last
                             else mybir.ActivationFunctionType.Relu),
                            bias=ct[("bcol", s)][:], scale=1.0)
                    nc.sync.dma_start(out_d[wl], hout[:])

    return _finish(nc)


def _finish(nc):
    nc.compile()
    return nc


# ------------------------------------------------------------ full pipeline

def make_in_maps(cfg, g, cc, xT_am_full, xT_ph_full, conv_am, conv_ph):
    """Build per-core input dicts for one layer launch."""
    maps = []
    for core in range(cfg.n_cores):
        m = dict(
            xT_am=rotate_tiles(xT_am_full, core, cfg),
            xT_ph=rotate_tiles(xT_ph_full, core, cfg),
            Wg_am=conv_am["Wg"], Tinv_am=conv_am["Tinv"],
            bcol_am=conv_am["bcol"], Wg_ph=conv_ph["Wg"],
            Tinv_ph=conv_ph["Tinv"], bcol_ph=conv_ph["bcol"],
            H2_am=cc["H2am"], H2_ph=cc["H2ph"], iotab=cc["iotab"],
            iotac=cc["iotac"], ident=cc["ident"], meta=g["meta"][core],
        )
        maps.append(m)
    return maps


def assemble(outs, cfg, oc, si):
    """per-core out (wpc, OC, 2, P) list -> (nw*P, OC) node-major fp32."""
    full = np.concatenate(
        [np.asarray(o)[:, :, si, :] for o in outs], 0)     # (nw, OC, P)
    return np.ascontiguousarray(
        full.transpose(0, 2, 1).astype(np.float32)).reshape(cfg.n_pad, oc)


def hT_tiles(outs, cfg, si):
    """layer-0 per-core outs -> full (128, nw, P) bf16 d-major tiles."""
    return np.ascontiguousarray(
        np.concatenate([np.asarray(o)[:, :, si, :] for o in outs], 0)
        .transpose(1, 0, 2).astype(NPBF))


def run_pipeline(inputs, cfg, runner):
    """runner(nc, in_maps) -> list of per-core {name: np.ndarray} outputs."""
    g = prep_graph(np.asarray(inputs["src"]), np.asarray(inputs["dst"]), cfg)
    cc = consts_np(cfg)
    fill_lnw(g, inputs["am_exist"], inputs["exist"], cfg)

    conv0a = prep_conv(inputs["W0a"], inputs["al0a"], inputs["ar0a"],
                       inputs["b0a"], False, cfg)
    conv0p = prep_conv(inputs["W0p"], inputs["al0p"], inputs["ar0p"],
                       inputs["b0p"], False, cfg)
    conv1a = prep_conv(inputs["W1a"], inputs["al1a"], inputs["ar1a"],
                       inputs["b1a"], True, cfg)
    conv1p = prep_conv(inputs["W1p"], inputs["al1p"], inputs["ar1p"],
                       inputs["b1p"], True, cfg)

    xT_am = to_xT_tiled(np.asarray(inputs["x_am"]), g, cfg)
    xT_ph = to_xT_tiled(np.asarray(inputs["x_ph"]), g, cfg)

    nc0 = build_layer(cfg, g["TA"], g["TB"], last=False)
    maps0 = make_in_maps(cfg, g, cc, xT_am, xT_ph, conv0a, conv0p)
    outs0 = runner(nc0, maps0)

    hT_am = hT_tiles([o["out"] for o in outs0], cfg, 0)
    hT_ph = hT_tiles([o["out"] for o in outs0], cfg, 1)

    nc1 = build_layer(cfg, g["TA"], g["TB"], last=True)
    maps1 = make_in_maps(cfg, g, cc, hT_am, hT_ph, conv1a, conv1p)
    outs1 = runner(nc1, maps1)

    oam = assemble([o["out"] for o in outs1], cfg, cfg.fdim, 0)
    oph = assemble([o["out"] for o in outs1], cfg, cfg.fdim, 1)
    res_am = np.zeros((cfg.n_nodes, cfg.fdim), np.float32)
    res_ph = np.zeros((cfg.n_nodes, cfg.fdim), np.float32)
    nid = g["new_id"][: cfg.n_nodes]
    res_am[:] = oam[nid]
    res_ph[:] = oph[nid]
    return res_am, res_ph


# ------------------------------------------------------------ timed runner

def run_layer_timed(nc, in_maps, n_cores, repeats=3):
    """Execute with device-resident inputs; returns (results, times)."""
    import time as _time
    import jax
    from jax.sharding import Mesh, PartitionSpec, NamedSharding
    from jax.experimental.shard_map import shard_map
    from concourse import bass2jax

    bass2jax.install_neuronx_cc_hook()
    part_name = (nc.partition_id_tensor.name
                 if nc.partition_id_tensor is not None else None)
    in_names, out_names, out_avals, zero_outs = [], [], [], []
    for alloc in nc.m.functions[0].allocations:
        if not isinstance(alloc, mybir.MemoryLocationSet):
            continue
        name = alloc.memorylocations[0].name
        if alloc.kind == "ExternalInput":
            if name != part_name:
                in_names.append(name)
        elif alloc.kind == "ExternalOutput":
            out_names.append(name)
            shape = tuple(alloc.tensor_shape)
            dtype = mybir.dt.np(alloc.dtype)
            out_avals.append(jax.core.ShapedArray(shape, dtype))
            zero_outs.append(np.zeros(shape, dtype))
    n_params = len(in_names)
    all_in = list(in_names + out_names)
    if part_name is not None:
        all_in.append(part_name)

    def _body(*args):
        operands = list(args)
        if part_name is not None:
            operands.append(bass2jax.partition_id_tensor())
        outs = bass2jax._bass_exec_p.bind(
            *operands, out_avals=tuple(out_avals), in_names=tuple(all_in),
            out_names=tuple(out_names), lowering_input_output_aliases=(),
            sim_require_finite=True, sim_require_nnan=True, nc=nc)
        return tuple(outs)

    devices = jax.devices()[:n_cores]
    mesh = Mesh(np.asarray(devices), ("core",))
    spec = PartitionSpec("core")
    nin = n_params + len(out_names)
    f = jax.jit(shard_map(_body, mesh=mesh, in_specs=(spec,) * nin,
                          out_specs=(spec,) * len(out_names), check_rep=False))
    concat_in = [np.concatenate([np.asarray(m[nm]) for m in in_maps], 0)
                 for nm in in_names]
    concat_zeros = [np.zeros((n_cores * z.shape[0], *z.shape[1:]), z.dtype)
                    for z in zero_outs]
    sh = NamedSharding(mesh, spec)
    dev_in = [jax.device_put(a, sh) for a in concat_in]
    dev_zero = [jax.device_put(a, sh) for a in concat_zeros]
    outs = f(*dev_in, *dev_zero)
    jax.block_until_ready(outs)
    ts = []
    for _ in range(repeats):
        t0 = _time.perf_counter()
        o2 = f(*dev_in, *dev_zero)
        jax.block_until_ready(o2)
        ts.append(_time.perf_counter() - t0)
    res = []
    for c in range(n_cores):
        res.append({nm: np.asarray(outs[i]).reshape(n_cores, *out_avals[i].shape)[c]
                    for i, nm in enumerate(out_names)})
    return res, ts


def baseline_overhead(n_cores, repeats=5):
    """Dispatch+network floor: trivial 8-core kernel timed the same way."""
    nc = bacc.Bacc("TRN2", target_bir_lowering=False, debug=False)
    x = nc.dram_tensor("x", [P, P], F32, kind="ExternalInput")
    y = nc.dram_tensor("y", [P, P], F32, kind="ExternalOutput")
    with tile.TileContext(nc) as tc:
        with tc.tile_pool(name="p", bufs=1) as p:
            t = p.tile([P, P], F32)
            nc.sync.dma_start(t[:], x[:])
            nc.scalar.mul(t[:], t[:], 2.0)
            nc.sync.dma_start(y[:], t[:])
    nc.compile()
    maps = [{"x": np.zeros((P, P), np.float32)} for _ in range(n_cores)]
    _, ts = run_layer_timed(nc, maps, n_cores, repeats=repeats)
    return min(ts)


# ------------------------------------------------------------ kernel entry

_PERF = {"exec_ns": 0.0, "launch_info": []}


def _hw_runner(cfg, measure):
    from concourse.bass_utils import run_bass_kernel_spmd

    def run(nc, in_maps):
        if measure:
            res, ts = run_layer_timed(nc, in_maps, cfg.n_cores, repeats=3)
            _PERF["launch_info"].append(min(ts))
            return res
        res = run_bass_kernel_spmd(nc, in_maps,
                                   core_ids=list(range(cfg.n_cores)))
        return res.results
    return run


def kernel(**inputs):
    """Full DUPLEX-GAT forward on 8 trn2 cores. Returns (h_am, h_ph)."""
    import os
    cfg = Cfg()
    measure = bool(int(os.environ.get("GAT_MEASURE", "0")))
    res_am, res_ph = run_pipeline(inputs, cfg, _hw_runner(cfg, measure))
    return res_am, res_ph
